# revision 6
# baseline (speedup 1.0000x reference)
"""MoE head (top-2 routing, swiglu MLP + vocab projection) on 8 Trainium2 cores.

Expert-parallel: one expert per NeuronCore. Routing (tiny: router scores +
top-k + stable dispatch sort) is replicated bitwise on host CPU with jax-cpu
(matching the fp32 reference); each core runs its expert's full MLP + vocab
projection over that expert's tokens.

Device layout ("layout A", transposed activations):
  every matmul keeps the weight tile stationary (fp16, fast weight-load
  path) and streams activations as the fp32r moving operand (full PE rate
  for free-dim >= 256). Activations live as [feature, token] so no on-chip
  transposes are needed anywhere; the top-2 gate is folded into the
  activations before the vocab projection, so the host combine is a pure
  scatter-add.

Weights are pre-transposed/pre-tiled on host so every DMA is a contiguous
[128, n*elem] block read.
"""

import os
import sys
import subprocess
import tempfile

import numpy as np

for _p in ("/opt/trn_rl_repo",):
    if os.path.isdir(_p) and _p not in sys.path:
        sys.path.insert(0, _p)

B, S, DIM = 2, 1024, 1024
N_EXPERTS, K = 8, 2
VOCAB = 16384
HIDDEN = DIM * 8 // 3            # 2730
HID_P = 2816                     # HIDDEN padded to 22*128
NQ = HID_P // 128                # 22 (lin,gate) row-pair tiles / phase-2 k-tiles
KD = DIM // 128                  # 8
NV = VOCAB // 128                # 128
N_CORES = 8

# Routing must make the same discrete top-k choices as the reference, which
# runs under jax on CPU; replicate it in a JAX_PLATFORMS=cpu subprocess
# (this process's jax backend is the axon/trn2 platform).
def _cpu_jax_env():
    env = dict(os.environ)
    # The axon sitecustomize boots the trn2 PJRT plugin in every subprocess
    # when this var is set, overriding JAX_PLATFORMS; drop it and point
    # PYTHONPATH at jax's site-packages directly.
    env.pop("TRN_TERMINAL_POOL_IPS", None)
    env["JAX_PLATFORMS"] = "cpu"
    try:
        import jax

        sp = os.path.dirname(os.path.dirname(jax.__file__))
        env["PYTHONPATH"] = sp + os.pathsep + env.get("PYTHONPATH", "")
    except Exception:
        pass
    return env


_ROUTE_SRC = r"""
import os, sys
os.environ["JAX_PLATFORMS"] = "cpu"
import numpy as np
d = sys.argv[1]
x = np.load(os.path.join(d, "x.npy"))
wr = np.load(os.path.join(d, "wr.npy"))
import jax, jax.numpy as jnp
scores = jnp.einsum("bsd,nd->bsn", jnp.asarray(x), jnp.asarray(wr))
c, ids = jax.lax.top_k(scores, 2)
w = jax.nn.softmax(c, axis=-1)
np.save(os.path.join(d, "ids.npy"), np.asarray(ids))
np.save(os.path.join(d, "w.npy"), np.asarray(w, dtype=np.float32))
"""


def _route(x, w_router):
    try:
        with tempfile.TemporaryDirectory() as d:
            np.save(os.path.join(d, "x.npy"), np.asarray(x, np.float32))
            np.save(os.path.join(d, "wr.npy"), np.asarray(w_router, np.float32))
            src = os.path.join(d, "route.py")
            with open(src, "w") as f:
                f.write(_ROUTE_SRC)
            env = _cpu_jax_env()
            subprocess.run(
                [sys.executable, src, d],
                check=True,
                env=env,
                timeout=900,
                capture_output=True,
            )
            ids = np.load(os.path.join(d, "ids.npy"))
            w = np.load(os.path.join(d, "w.npy"))
            return ids, w
    except Exception:
        # numpy fallback replicating jax.lax.top_k tie semantics (lower
        # index wins on equal values).
        s = x.reshape(-1, DIM).astype(np.float32) @ w_router.astype(np.float32).T
        idx = np.argsort(-s, axis=-1, kind="stable")[:, :K]
        c = np.take_along_axis(s, idx, axis=-1)
        e = np.exp(c - c.max(-1, keepdims=True))
        w = e / e.sum(-1, keepdims=True)
        return (
            idx.reshape(B, S, K).astype(np.int32),
            w.reshape(B, S, K).astype(np.float32),
        )


def _build(C, chunks):
    import concourse.bacc as bacc
    import concourse.tile as tile
    import concourse.mybir as mybir

    f32 = mybir.dt.float32
    f16 = mybir.dt.float16
    SIGMOID = mybir.ActivationFunctionType.Sigmoid

    nc = bacc.Bacc("TRN2", target_bir_lowering=False, debug=False)

    xd = nc.dram_tensor("xt", [KD, 128, C], f16, kind="ExternalInput").ap()
    gd = nc.dram_tensor("g", [128, C], f16, kind="ExternalInput").ap()
    wud = nc.dram_tensor("wup", [NQ, 128, 2 * DIM], f16, kind="ExternalInput").ap()
    wdd = nc.dram_tensor("wdn", [KD, 128, HID_P], f16, kind="ExternalInput").ap()
    wpd = nc.dram_tensor("wpj", [NV, 128, DIM], f16, kind="ExternalInput").ap()
    ld = nc.dram_tensor("L", [VOCAB, C], f32, kind="ExternalOutput").ap()

    with tile.TileContext(nc) as tc:
        with tc.tile_pool(name="persist", bufs=1) as per:
            X = [per.tile([128, C], f16, name=f"X{j}") for j in range(KD)]
            G = per.tile([128, C], f16, name="G")
            A = [per.tile([128, C], f16, name=f"A{q}") for q in range(NQ)]
            Y = [per.tile([128, C], f16, name=f"Y{j}") for j in range(KD)]
            zb = per.tile([128, 1], f32, name="zb")
            nc.gpsimd.memset(zb[:], 0.0)
            for j in range(KD):
                nc.sync.dma_start(X[j][:], xd[j])
            nc.sync.dma_start(G[:], gd)

            # phase 1: H = Wup @ X  (per 128-row lin/gate pair), A = lin*silu(gate)
            with (
                tc.tile_pool(name="ph1w", bufs=3) as wp1,
                tc.tile_pool(name="ph1p", bufs=3, space="PSUM") as ps1,
                tc.tile_pool(name="ph1t", bufs=3) as tp1,
            ):
                for q in range(NQ):
                    wt = wp1.tile([128, 2 * DIM], f16, name="wt")
                    nc.sync.dma_start(wt[:], wud[q])
                    for (c0, c1) in chunks:
                        n = c1 - c0
                        pl = ps1.tile([128, n], f32, name="pl")
                        pg = ps1.tile([128, n], f32, name="pg")
                        for j in range(KD):
                            rhs = X[j][:, c0:c1]
                            nc.tensor.matmul(
                                pl[:], wt[:, j * 256 : j * 256 + 128], rhs,
                                start=(j == 0), stop=(j == KD - 1),
                            )
                            nc.tensor.matmul(
                                pg[:], wt[:, j * 256 + 128 : (j + 1) * 256], rhs,
                                start=(j == 0), stop=(j == KD - 1),
                            )
                        st = tp1.tile([128, n], f32, name="st")
                        nc.scalar.activation(st[:], pg[:], SIGMOID, bias=zb[:])
                        nc.vector.tensor_mul(st[:], st[:], pg[:])
                        nc.vector.tensor_mul(A[q][:, c0:c1], pl[:], st[:])

            # phase 2: Y = (Wdown @ A + X) * gate
            with (
                tc.tile_pool(name="ph2w", bufs=2) as wp2,
                tc.tile_pool(name="ph2p", bufs=4, space="PSUM") as ps2,
            ):
                for m in range(KD):
                    wd = wp2.tile([128, HID_P], f16, name="wd")
                    nc.sync.dma_start(wd[:], wdd[m])
                    for (c0, c1) in chunks:
                        n = c1 - c0
                        py = ps2.tile([128, n], f32, name="py")
                        for j in range(NQ):
                            nc.tensor.matmul(
                                py[:], wd[:, j * 128 : (j + 1) * 128],
                                A[j][:, c0:c1],
                                start=(j == 0), stop=(j == NQ - 1),
                            )
                        nc.vector.tensor_add(py[:], py[:], X[m][:, c0:c1])
                        nc.vector.tensor_mul(Y[m][:, c0:c1], py[:], G[:, c0:c1])

            # phase 3: L = Wproj @ Y
            with (
                tc.tile_pool(name="ph3w", bufs=4) as wp3,
                tc.tile_pool(name="ph3p", bufs=8, space="PSUM") as ps3,
                tc.tile_pool(name="ph3o", bufs=4) as op3,
            ):
                for v in range(NV):
                    wp = wp3.tile([128, DIM], f16, name="wp")
                    nc.sync.dma_start(wp[:], wpd[v])
                    for (c0, c1) in chunks:
                        n = c1 - c0
                        pL = ps3.tile([128, n], f32, name="pL")
                        for j in range(KD):
                            nc.tensor.matmul(
                                pL[:], wp[:, j * 128 : (j + 1) * 128],
                                Y[j][:, c0:c1],
                                start=(j == 0), stop=(j == KD - 1),
                            )
                        ot = op3.tile([128, n], f32, name="ot")
                        nc.vector.tensor_copy(ot[:], pL[:])
                        nc.sync.dma_start(ld[v * 128 : (v + 1) * 128, c0:c1], ot[:])

    nc.compile()
    return nc


def _prep_core_inputs(e, x_flat, w_up, w_down, w_proj, tok, gates, C):
    cnt = len(tok)
    Xp = np.zeros((C, DIM), np.float32)
    if cnt:
        Xp[:cnt] = x_flat[tok]
    xt = np.ascontiguousarray(Xp.T.reshape(KD, 128, C)).astype(np.float16)

    g = np.zeros((C,), np.float32)
    if cnt:
        g[:cnt] = gates
    gb = np.ascontiguousarray(np.broadcast_to(g, (128, C))).astype(np.float16)

    wu = np.asarray(w_up[e], np.float32)
    lin = np.zeros((HID_P, DIM), np.float32)
    lin[:HIDDEN] = wu[:HIDDEN]
    gat = np.zeros((HID_P, DIM), np.float32)
    gat[:HIDDEN] = wu[HIDDEN : 2 * HIDDEN]
    ilv = np.empty((NQ, 2, 128, DIM), np.float32)
    ilv[:, 0] = lin.reshape(NQ, 128, DIM)
    ilv[:, 1] = gat.reshape(NQ, 128, DIM)
    # [q, p, j, c] = row(q*256+c) of interleaved, col (j*128+p)
    wup_t = np.ascontiguousarray(
        ilv.reshape(NQ * 256, DIM).reshape(NQ, 256, KD, 128).transpose(0, 3, 2, 1)
    ).astype(np.float16).reshape(NQ, 128, 2 * DIM)

    wdt = np.zeros((HID_P, DIM), np.float32)
    wdt[:HIDDEN] = np.asarray(w_down[e], np.float32).T
    wdn_t = np.ascontiguousarray(
        wdt.reshape(NQ, 128, KD, 128).transpose(2, 1, 0, 3)
    ).astype(np.float16).reshape(KD, 128, HID_P)

    wpj_t = np.ascontiguousarray(
        np.asarray(w_proj[e], np.float32)
        .reshape(NV, 128, KD, 128)
        .transpose(0, 3, 2, 1)
    ).astype(np.float16).reshape(NV, 128, DIM)

    return {"xt": xt, "g": gb, "wup": wup_t, "wdn": wdn_t, "wpj": wpj_t}


_last_results = None  # for test harness inspection (exec_time_ns etc.)


def kernel(x, w_router, w_up, w_down, w_proj):
    global _last_results
    x = np.asarray(x, np.float32)

    ids, wsm = _route(x, w_router)
    ids_flat = ids.reshape(-1).astype(np.int64)
    w_flat = wsm.reshape(-1).astype(np.float32)
    order = np.argsort(ids_flat, kind="stable")
    counts = np.bincount(ids_flat, minlength=N_EXPERTS)
    offs = np.concatenate([[0], np.cumsum(counts)])

    C = int(counts.max())
    n_ch = max(1, -(-C // 512))
    base, rem = divmod(C, n_ch)
    sizes = [base + (1 if i < rem else 0) for i in range(n_ch)]
    chunks = []
    o = 0
    for s_ in sizes:
        chunks.append((o, o + s_))
        o += s_

    x_flat = x.reshape(B * S, DIM)
    in_maps = []
    tok_lists = []
    for e in range(N_EXPERTS):
        rows = order[offs[e] : offs[e + 1]]
        tok = rows // K
        tok_lists.append(tok)
        in_maps.append(
            _prep_core_inputs(e, x_flat, w_up, w_down, w_proj, tok, w_flat[rows], C)
        )

    nc = _build(C, chunks)

    from concourse.bass_utils import run_bass_kernel_spmd

    trace = bool(int(os.environ.get("MOE_KERNEL_TRACE", "0")))
    kw = {}
    if trace:
        kw["trace"] = True
        kw["trace_cores"] = list(range(N_CORES))
    res = run_bass_kernel_spmd(nc, in_maps, list(range(N_CORES)), **kw)
    _last_results = res

    out_flat = np.zeros((B * S, VOCAB), np.float32)
    for e in range(N_EXPERTS):
        tok = tok_lists[e]
        cnt = len(tok)
        if cnt:
            out_flat[tok] += res.results[e]["L"][:, :cnt].T
    return out_flat.reshape(B, S, VOCAB)


# revision 9
# speedup vs baseline: 1.2329x; 1.2329x over previous
"""MoE head (top-2 routing, swiglu MLP + vocab projection) on 8 Trainium2 cores.

Expert-parallel: one expert per NeuronCore. Routing (tiny: router scores +
top-k + stable dispatch sort) is replicated bitwise on host CPU with jax-cpu
(matching the fp32 reference); each core runs its expert's full MLP + vocab
projection over that expert's tokens.

Device layout ("layout A", transposed activations):
  every matmul keeps the weight tile stationary (fp16, fast weight-load
  path) and streams activations as the fp32r moving operand (full PE rate
  for free-dim >= 256). Activations live as [feature, token] so no on-chip
  transposes are needed anywhere; the top-2 gate is folded into the
  activations before the vocab projection, so the host combine is a pure
  scatter-add.

Weights are pre-transposed/pre-tiled on host so every DMA is a contiguous
[128, n*elem] block read.
"""

import os
import sys
import subprocess
import tempfile

import numpy as np

for _p in ("/opt/trn_rl_repo",):
    if os.path.isdir(_p) and _p not in sys.path:
        sys.path.insert(0, _p)

B, S, DIM = 2, 1024, 1024
N_EXPERTS, K = 8, 2
VOCAB = 16384
HIDDEN = DIM * 8 // 3            # 2730
HID_P = 2816                     # HIDDEN padded to 22*128
NQ = HID_P // 128                # 22 (lin,gate) row-pair tiles / phase-2 k-tiles
KD = DIM // 128                  # 8
NV = VOCAB // 128                # 128
N_CORES = 8

# Routing must make the same discrete top-k choices as the reference, which
# runs under jax on CPU; replicate it in a JAX_PLATFORMS=cpu subprocess
# (this process's jax backend is the axon/trn2 platform).
def _cpu_jax_env():
    env = dict(os.environ)
    # The axon sitecustomize boots the trn2 PJRT plugin in every subprocess
    # when this var is set, overriding JAX_PLATFORMS; drop it and point
    # PYTHONPATH at jax's site-packages directly.
    env.pop("TRN_TERMINAL_POOL_IPS", None)
    env["JAX_PLATFORMS"] = "cpu"
    try:
        import jax

        sp = os.path.dirname(os.path.dirname(jax.__file__))
        env["PYTHONPATH"] = sp + os.pathsep + env.get("PYTHONPATH", "")
    except Exception:
        pass
    return env


_ROUTE_SRC = r"""
import os, sys
os.environ["JAX_PLATFORMS"] = "cpu"
import numpy as np
d = sys.argv[1]
x = np.load(os.path.join(d, "x.npy"))
wr = np.load(os.path.join(d, "wr.npy"))
import jax, jax.numpy as jnp
scores = jnp.einsum("bsd,nd->bsn", jnp.asarray(x), jnp.asarray(wr))
c, ids = jax.lax.top_k(scores, 2)
w = jax.nn.softmax(c, axis=-1)
np.save(os.path.join(d, "ids.npy"), np.asarray(ids))
np.save(os.path.join(d, "w.npy"), np.asarray(w, dtype=np.float32))
"""


def _route(x, w_router):
    try:
        with tempfile.TemporaryDirectory() as d:
            np.save(os.path.join(d, "x.npy"), np.asarray(x, np.float32))
            np.save(os.path.join(d, "wr.npy"), np.asarray(w_router, np.float32))
            src = os.path.join(d, "route.py")
            with open(src, "w") as f:
                f.write(_ROUTE_SRC)
            env = _cpu_jax_env()
            subprocess.run(
                [sys.executable, src, d],
                check=True,
                env=env,
                timeout=900,
                capture_output=True,
            )
            ids = np.load(os.path.join(d, "ids.npy"))
            w = np.load(os.path.join(d, "w.npy"))
            return ids, w
    except Exception:
        # numpy fallback replicating jax.lax.top_k tie semantics (lower
        # index wins on equal values).
        s = x.reshape(-1, DIM).astype(np.float32) @ w_router.astype(np.float32).T
        idx = np.argsort(-s, axis=-1, kind="stable")[:, :K]
        c = np.take_along_axis(s, idx, axis=-1)
        e = np.exp(c - c.max(-1, keepdims=True))
        w = e / e.sum(-1, keepdims=True)
        return (
            idx.reshape(B, S, K).astype(np.int32),
            w.reshape(B, S, K).astype(np.float32),
        )


def _build(C, chunks):
    import concourse.bacc as bacc
    import concourse.tile as tile
    import concourse.mybir as mybir

    f32 = mybir.dt.float32
    f16 = mybir.dt.float16
    SIGMOID = mybir.ActivationFunctionType.Sigmoid

    nc = bacc.Bacc("TRN2", target_bir_lowering=False, debug=False)

    xd = nc.dram_tensor("xt", [KD, 128, C], f16, kind="ExternalInput").ap()
    gd = nc.dram_tensor("g", [128, C], f16, kind="ExternalInput").ap()
    wud = nc.dram_tensor("wup", [NQ, 128, 2 * DIM], f16, kind="ExternalInput").ap()
    wdd = nc.dram_tensor("wdn", [KD, 128, HID_P], f16, kind="ExternalInput").ap()
    # wproj pre-tiled as 64 pairs of v-tiles: one [128, 2048] DMA = 2 v-tiles
    wpd = nc.dram_tensor("wpj", [NV // 2, 128, 2 * DIM], f16, kind="ExternalInput").ap()
    ld = nc.dram_tensor("L", [VOCAB, C], f16, kind="ExternalOutput").ap()

    with tile.TileContext(nc) as tc:
        with (
            tc.tile_pool(name="persist", bufs=1) as per,
            tc.tile_pool(name="wpool", bufs=1) as wpool,
            tc.tile_pool(name="tpool", bufs=1) as tpool,
            tc.tile_pool(name="pspool", bufs=1, space="PSUM") as ps,
        ):
            # weight-tile DMA for the very first phase-1 pair goes first so
            # the PE can start as early as possible.
            wt0 = wpool.tile([128, 2 * DIM], f16, name="wt", tag="wt", bufs=3)
            nc.sync.dma_start(wt0[:], wud[0])
            X = [per.tile([128, C], f16, name=f"X{j}") for j in range(KD)]
            G = per.tile([128, C], f16, name="G")
            A = [per.tile([128, C], f16, name=f"A{q}") for q in range(NQ)]
            Y = [per.tile([128, C], f16, name=f"Y{j}") for j in range(KD)]
            zb = per.tile([128, 1], f32, name="zb")
            nc.gpsimd.memset(zb[:], 0.0)
            for j in range(KD):
                nc.sync.dma_start(X[j][:], xd[j])
            nc.sync.dma_start(G[:], gd)

            # phase 1: H = Wup @ X  (per 128-row lin/gate pair), A = lin*silu(gate)
            for q in range(NQ):
                if q == 0:
                    wt = wt0
                else:
                    wt = wpool.tile([128, 2 * DIM], f16, name="wt", tag="wt", bufs=3)
                    nc.sync.dma_start(wt[:], wud[q])
                for (c0, c1) in chunks:
                    n = c1 - c0
                    pl = ps.tile([128, 512], f32, name="pl", tag="pl", bufs=2)
                    pg = ps.tile([128, 512], f32, name="pg", tag="pg", bufs=2)
                    for j in range(KD):
                        rhs = X[j][:, c0:c1]
                        nc.tensor.matmul(
                            pl[:, :n], wt[:, j * 256 : j * 256 + 128], rhs,
                            start=(j == 0), stop=(j == KD - 1),
                        )
                        nc.tensor.matmul(
                            pg[:, :n], wt[:, j * 256 + 128 : (j + 1) * 256], rhs,
                            start=(j == 0), stop=(j == KD - 1),
                        )
                    st = tpool.tile([128, 512], f32, name="st", tag="st", bufs=3)
                    nc.scalar.activation(st[:, :n], pg[:, :n], SIGMOID, bias=zb[:])
                    nc.vector.tensor_mul(st[:, :n], st[:, :n], pg[:, :n])
                    nc.vector.tensor_mul(A[q][:, c0:c1], pl[:, :n], st[:, :n])

            # phase 2: Y = (Wdown @ A + X) * gate
            for m in range(KD):
                wd = wpool.tile([128, HID_P], f16, name="wd", tag="wd", bufs=2)
                nc.sync.dma_start(wd[:], wdd[m])
                for (c0, c1) in chunks:
                    n = c1 - c0
                    py = ps.tile([128, 512], f32, name="py", tag="py", bufs=2)
                    for j in range(NQ):
                        nc.tensor.matmul(
                            py[:, :n], wd[:, j * 128 : (j + 1) * 128],
                            A[j][:, c0:c1],
                            start=(j == 0), stop=(j == NQ - 1),
                        )
                    nc.vector.tensor_add(py[:, :n], py[:, :n], X[m][:, c0:c1])
                    nc.vector.tensor_mul(Y[m][:, c0:c1], py[:, :n], G[:, c0:c1])

            # phase 3: L = Wproj @ Y, two v-tiles per weight DMA, one out
            # tile + DMA per v-tile (both chunks batched)
            for vp in range(NV // 2):
                wp = wpool.tile([128, 2 * DIM], f16, name="wp", tag="wp", bufs=4)
                nc.sync.dma_start(wp[:], wpd[vp])
                for h in range(2):
                    v = 2 * vp + h
                    wv = wp[:, h * DIM : (h + 1) * DIM]
                    ot = tpool.tile([128, C], f16, name="ot", tag="ot", bufs=4)
                    for (c0, c1) in chunks:
                        n = c1 - c0
                        pL = ps.tile([128, 512], f32, name="pL", tag="pL", bufs=2)
                        for j in range(KD):
                            nc.tensor.matmul(
                                pL[:, :n], wv[:, j * 128 : (j + 1) * 128],
                                Y[j][:, c0:c1],
                                start=(j == 0), stop=(j == KD - 1),
                            )
                        nc.vector.tensor_copy(ot[:, c0:c1], pL[:, :n])
                    nc.sync.dma_start(ld[v * 128 : (v + 1) * 128, :], ot[:])

    nc.compile()
    return nc


def _prep_core_inputs(e, x_flat, w_up, w_down, w_proj, tok, gates, C):
    cnt = len(tok)
    Xp = np.zeros((C, DIM), np.float32)
    if cnt:
        Xp[:cnt] = x_flat[tok]
    xt = np.ascontiguousarray(Xp.T.reshape(KD, 128, C)).astype(np.float16)

    g = np.zeros((C,), np.float32)
    if cnt:
        g[:cnt] = gates
    gb = np.ascontiguousarray(np.broadcast_to(g, (128, C))).astype(np.float16)

    wu = np.asarray(w_up[e], np.float32)
    lin = np.zeros((HID_P, DIM), np.float32)
    lin[:HIDDEN] = wu[:HIDDEN]
    gat = np.zeros((HID_P, DIM), np.float32)
    gat[:HIDDEN] = wu[HIDDEN : 2 * HIDDEN]
    ilv = np.empty((NQ, 2, 128, DIM), np.float32)
    ilv[:, 0] = lin.reshape(NQ, 128, DIM)
    ilv[:, 1] = gat.reshape(NQ, 128, DIM)
    # [q, p, j, c] = row(q*256+c) of interleaved, col (j*128+p)
    wup_t = np.ascontiguousarray(
        ilv.reshape(NQ * 256, DIM).reshape(NQ, 256, KD, 128).transpose(0, 3, 2, 1)
    ).astype(np.float16).reshape(NQ, 128, 2 * DIM)

    wdt = np.zeros((HID_P, DIM), np.float32)
    wdt[:HIDDEN] = np.asarray(w_down[e], np.float32).T
    wdn_t = np.ascontiguousarray(
        wdt.reshape(NQ, 128, KD, 128).transpose(2, 1, 0, 3)
    ).astype(np.float16).reshape(KD, 128, HID_P)

    wpj_t = (
        np.ascontiguousarray(
            np.asarray(w_proj[e], np.float32)
            .reshape(NV, 128, KD, 128)
            .transpose(0, 3, 2, 1)
        )
        .astype(np.float16)
        .reshape(NV // 2, 2, 128, DIM)
        .transpose(0, 2, 1, 3)
        .reshape(NV // 2, 128, 2 * DIM)
    )
    wpj_t = np.ascontiguousarray(wpj_t)

    return {"xt": xt, "g": gb, "wup": wup_t, "wdn": wdn_t, "wpj": wpj_t}


_last_results = None  # for test harness inspection (exec_time_ns etc.)


def kernel(x, w_router, w_up, w_down, w_proj):
    global _last_results
    x = np.asarray(x, np.float32)

    ids, wsm = _route(x, w_router)
    ids_flat = ids.reshape(-1).astype(np.int64)
    w_flat = wsm.reshape(-1).astype(np.float32)
    order = np.argsort(ids_flat, kind="stable")
    counts = np.bincount(ids_flat, minlength=N_EXPERTS)
    offs = np.concatenate([[0], np.cumsum(counts)])

    C = int(counts.max())
    n_ch = max(1, -(-C // 512))
    base, rem = divmod(C, n_ch)
    sizes = [base + (1 if i < rem else 0) for i in range(n_ch)]
    chunks = []
    o = 0
    for s_ in sizes:
        chunks.append((o, o + s_))
        o += s_

    x_flat = x.reshape(B * S, DIM)
    in_maps = []
    tok_lists = []
    for e in range(N_EXPERTS):
        rows = order[offs[e] : offs[e + 1]]
        tok = rows // K
        tok_lists.append(tok)
        in_maps.append(
            _prep_core_inputs(e, x_flat, w_up, w_down, w_proj, tok, w_flat[rows], C)
        )

    nc = _build(C, chunks)

    from concourse.bass_utils import run_bass_kernel_spmd

    trace = bool(int(os.environ.get("MOE_KERNEL_TRACE", "0")))
    kw = {}
    if trace:
        kw["trace"] = True
        kw["trace_cores"] = list(range(N_CORES))
    res = run_bass_kernel_spmd(nc, in_maps, list(range(N_CORES)), **kw)
    _last_results = res

    out_flat = np.zeros((B * S, VOCAB), np.float32)
    for e in range(N_EXPERTS):
        tok = tok_lists[e]
        cnt = len(tok)
        if cnt:
            out_flat[tok] += res.results[e]["L"][:, :cnt].T.astype(np.float32)
    return out_flat.reshape(B, S, VOCAB)


# revision 11
# speedup vs baseline: 1.3708x; 1.1119x over previous
"""MoE head (top-2 routing, swiglu MLP + vocab projection) on 8 Trainium2 cores.

Expert-parallel: one expert per NeuronCore. Routing (tiny: router scores +
top-k + stable dispatch sort) is replicated bitwise on host CPU with jax-cpu
(matching the fp32 reference); each core runs its expert's full MLP + vocab
projection over that expert's tokens.

Device layout ("layout A", transposed activations): every matmul keeps a
weight tile stationary and streams activations as the moving operand; all
matmul operands are fp16 (PE upconverts to ~FP22 internally, fp32 PSUM
accumulate), which sustains the pure streaming rate (N cols / PE clock per
matmul, self-loading weight loads fully hidden). Activations live as
[feature, token] so no on-chip transposes are needed anywhere; the top-2
gate is folded into the activations before the vocab projection, so the
host combine is a pure scatter-add.

Weights are pre-transposed/pre-tiled on host so every DMA is a contiguous
[128, n*elem] block read.
"""

import os
import sys
import subprocess
import tempfile

import numpy as np

for _p in ("/opt/trn_rl_repo",):
    if os.path.isdir(_p) and _p not in sys.path:
        sys.path.insert(0, _p)

B, S, DIM = 2, 1024, 1024
N_EXPERTS, K = 8, 2
VOCAB = 16384
HIDDEN = DIM * 8 // 3            # 2730
HID_P = 2816                     # HIDDEN padded to 22*128
NQ = HID_P // 128                # 22 (lin,gate) row-pair tiles / phase-2 k-tiles
KD = DIM // 128                  # 8
NV = VOCAB // 128                # 128
N_CORES = 8

# Routing must make the same discrete top-k choices as the reference, which
# runs under jax on CPU; replicate it in a JAX_PLATFORMS=cpu subprocess
# (this process's jax backend is the axon/trn2 platform).
def _cpu_jax_env():
    env = dict(os.environ)
    # The axon sitecustomize boots the trn2 PJRT plugin in every subprocess
    # when this var is set, overriding JAX_PLATFORMS; drop it and point
    # PYTHONPATH at jax's site-packages directly.
    env.pop("TRN_TERMINAL_POOL_IPS", None)
    env["JAX_PLATFORMS"] = "cpu"
    try:
        import jax

        sp = os.path.dirname(os.path.dirname(jax.__file__))
        env["PYTHONPATH"] = sp + os.pathsep + env.get("PYTHONPATH", "")
    except Exception:
        pass
    return env


_ROUTE_SRC = r"""
import os, sys
os.environ["JAX_PLATFORMS"] = "cpu"
import numpy as np
d = sys.argv[1]
x = np.load(os.path.join(d, "x.npy"))
wr = np.load(os.path.join(d, "wr.npy"))
import jax, jax.numpy as jnp
scores = jnp.einsum("bsd,nd->bsn", jnp.asarray(x), jnp.asarray(wr))
c, ids = jax.lax.top_k(scores, 2)
w = jax.nn.softmax(c, axis=-1)
np.save(os.path.join(d, "ids.npy"), np.asarray(ids))
np.save(os.path.join(d, "w.npy"), np.asarray(w, dtype=np.float32))
"""


def _route(x, w_router):
    try:
        with tempfile.TemporaryDirectory() as d:
            np.save(os.path.join(d, "x.npy"), np.asarray(x, np.float32))
            np.save(os.path.join(d, "wr.npy"), np.asarray(w_router, np.float32))
            src = os.path.join(d, "route.py")
            with open(src, "w") as f:
                f.write(_ROUTE_SRC)
            env = _cpu_jax_env()
            subprocess.run(
                [sys.executable, src, d],
                check=True,
                env=env,
                timeout=900,
                capture_output=True,
            )
            ids = np.load(os.path.join(d, "ids.npy"))
            w = np.load(os.path.join(d, "w.npy"))
            return ids, w
    except Exception:
        # numpy fallback replicating jax.lax.top_k tie semantics (lower
        # index wins on equal values).
        s = x.reshape(-1, DIM).astype(np.float32) @ w_router.astype(np.float32).T
        idx = np.argsort(-s, axis=-1, kind="stable")[:, :K]
        c = np.take_along_axis(s, idx, axis=-1)
        e = np.exp(c - c.max(-1, keepdims=True))
        w = e / e.sum(-1, keepdims=True)
        return (
            idx.reshape(B, S, K).astype(np.int32),
            w.reshape(B, S, K).astype(np.float32),
        )


def _build(C, chunks):
    import concourse.bacc as bacc
    import concourse.tile as tile
    import concourse.mybir as mybir

    f32 = mybir.dt.float32
    f16 = mybir.dt.float16
    SIGMOID = mybir.ActivationFunctionType.Sigmoid

    # SBUF guard: activation tiles scale with C; shrink buffering for very
    # skewed routing (C is ~547 for the reference inputs).
    big = C > 1500
    WT_BUFS = 3 if big else 4
    WP_BUFS = 4 if big else 6
    OT_BUFS = 2 if big else 4

    nc = bacc.Bacc("TRN2", target_bir_lowering=False, debug=False)

    xd = nc.dram_tensor("xt", [KD, 128, C], f16, kind="ExternalInput").ap()
    gd = nc.dram_tensor("g", [128, C], f16, kind="ExternalInput").ap()
    wud = nc.dram_tensor("wup", [NQ, 128, 2 * DIM], f16, kind="ExternalInput").ap()
    wdd = nc.dram_tensor("wdn", [KD, 128, HID_P], f16, kind="ExternalInput").ap()
    # wproj pre-tiled as 64 pairs of v-tiles: one [128, 2048] DMA = 2 v-tiles
    wpd = nc.dram_tensor("wpj", [NV // 2, 128, 2 * DIM], f16, kind="ExternalInput").ap()
    ld = nc.dram_tensor("L", [VOCAB, C], f16, kind="ExternalOutput").ap()

    with tile.TileContext(nc) as tc:
        with (
            tc.tile_pool(name="persist", bufs=1) as per,
            tc.tile_pool(name="wpool", bufs=1) as wpool,
            tc.tile_pool(name="tpool", bufs=1) as tpool,
            tc.tile_pool(name="pspool", bufs=1, space="PSUM") as ps,
        ):
            # weight-tile DMA for the very first phase-1 pair goes first so
            # the PE can start as early as possible.
            wt0 = wpool.tile([128, 2 * DIM], f16, name="wt", tag="wt", bufs=WT_BUFS)
            nc.sync.dma_start(wt0[:], wud[0])
            X = [per.tile([128, C], f16, name=f"X{j}") for j in range(KD)]
            G = per.tile([128, C], f16, name="G")
            A = [per.tile([128, C], f16, name=f"A{q}") for q in range(NQ)]
            Y = [per.tile([128, C], f16, name=f"Y{j}") for j in range(KD)]
            zb = per.tile([128, 1], f32, name="zb")
            nc.gpsimd.memset(zb[:], 0.0)
            for j in range(KD):
                nc.sync.dma_start(X[j][:], xd[j])
            nc.sync.dma_start(G[:], gd)

            # phase 1: H = Wup @ X  (per 128-row lin/gate pair), A = lin*silu(gate)
            for q in range(NQ):
                if q == 0:
                    wt = wt0
                else:
                    wt = wpool.tile([128, 2 * DIM], f16, name="wt", tag="wt", bufs=WT_BUFS)
                    nc.sync.dma_start(wt[:], wud[q])
                for (c0, c1) in chunks:
                    n = c1 - c0
                    pl = ps.tile([128, 512], f32, name="pl", tag="pl", bufs=2)
                    pg = ps.tile([128, 512], f32, name="pg", tag="pg", bufs=2)
                    for j in range(KD):
                        rhs = X[j][:, c0:c1]
                        nc.tensor.matmul(
                            pl[:, :n], wt[:, j * 256 : j * 256 + 128], rhs,
                            start=(j == 0), stop=(j == KD - 1),
                        )
                        nc.tensor.matmul(
                            pg[:, :n], wt[:, j * 256 + 128 : (j + 1) * 256], rhs,
                            start=(j == 0), stop=(j == KD - 1),
                        )
                    st = tpool.tile([128, 512], f32, name="st", tag="st", bufs=3)
                    nc.scalar.activation(st[:, :n], pg[:, :n], SIGMOID, bias=zb[:])
                    nc.vector.tensor_mul(st[:, :n], st[:, :n], pg[:, :n])
                    nc.vector.tensor_mul(A[q][:, c0:c1], pl[:, :n], st[:, :n])

            # phase 2: Y = (Wdown @ A + X) * gate
            for m in range(KD):
                wd = wpool.tile([128, HID_P], f16, name="wd", tag="wd", bufs=2)
                nc.sync.dma_start(wd[:], wdd[m])
                for (c0, c1) in chunks:
                    n = c1 - c0
                    py = ps.tile([128, 512], f32, name="py", tag="py", bufs=2)
                    for j in range(NQ):
                        nc.tensor.matmul(
                            py[:, :n], wd[:, j * 128 : (j + 1) * 128],
                            A[j][:, c0:c1],
                            start=(j == 0), stop=(j == NQ - 1),
                        )
                    nc.vector.tensor_add(py[:, :n], py[:, :n], X[m][:, c0:c1])
                    nc.vector.tensor_mul(Y[m][:, c0:c1], py[:, :n], G[:, c0:c1])

            # phase 3: L = Wproj @ Y, two v-tiles per weight DMA, one out
            # tile + DMA per v-tile (both chunks batched)
            for vp in range(NV // 2):
                wp = wpool.tile([128, 2 * DIM], f16, name="wp", tag="wp", bufs=WP_BUFS)
                nc.sync.dma_start(wp[:], wpd[vp])
                for h in range(2):
                    v = 2 * vp + h
                    wv = wp[:, h * DIM : (h + 1) * DIM]
                    ot = tpool.tile([128, C], f16, name="ot", tag="ot", bufs=OT_BUFS)
                    for (c0, c1) in chunks:
                        n = c1 - c0
                        pL = ps.tile([128, 512], f32, name="pL", tag="pL", bufs=2)
                        for j in range(KD):
                            nc.tensor.matmul(
                                pL[:, :n], wv[:, j * 128 : (j + 1) * 128],
                                Y[j][:, c0:c1],
                                start=(j == 0), stop=(j == KD - 1),
                            )
                        nc.vector.tensor_copy(ot[:, c0:c1], pL[:, :n])
                    nc.scalar.dma_start(ld[v * 128 : (v + 1) * 128, :], ot[:])

    nc.compile()
    return nc


def _prep_core_inputs(e, x_flat, w_up, w_down, w_proj, tok, gates, C):
    cnt = len(tok)
    Xp = np.zeros((C, DIM), np.float32)
    if cnt:
        Xp[:cnt] = x_flat[tok]
    xt = np.ascontiguousarray(Xp.T.reshape(KD, 128, C)).astype(np.float16)

    g = np.zeros((C,), np.float32)
    if cnt:
        g[:cnt] = gates
    gb = np.ascontiguousarray(np.broadcast_to(g, (128, C))).astype(np.float16)

    wu = np.asarray(w_up[e], np.float32)
    lin = np.zeros((HID_P, DIM), np.float32)
    lin[:HIDDEN] = wu[:HIDDEN]
    gat = np.zeros((HID_P, DIM), np.float32)
    gat[:HIDDEN] = wu[HIDDEN : 2 * HIDDEN]
    ilv = np.empty((NQ, 2, 128, DIM), np.float32)
    ilv[:, 0] = lin.reshape(NQ, 128, DIM)
    ilv[:, 1] = gat.reshape(NQ, 128, DIM)
    # [q, p, j, c] = row(q*256+c) of interleaved, col (j*128+p)
    wup_t = np.ascontiguousarray(
        ilv.reshape(NQ * 256, DIM).reshape(NQ, 256, KD, 128).transpose(0, 3, 2, 1)
    ).astype(np.float16).reshape(NQ, 128, 2 * DIM)

    wdt = np.zeros((HID_P, DIM), np.float32)
    wdt[:HIDDEN] = np.asarray(w_down[e], np.float32).T
    wdn_t = np.ascontiguousarray(
        wdt.reshape(NQ, 128, KD, 128).transpose(2, 1, 0, 3)
    ).astype(np.float16).reshape(KD, 128, HID_P)

    wpj_t = (
        np.ascontiguousarray(
            np.asarray(w_proj[e], np.float32)
            .reshape(NV, 128, KD, 128)
            .transpose(0, 3, 2, 1)
        )
        .astype(np.float16)
        .reshape(NV // 2, 2, 128, DIM)
        .transpose(0, 2, 1, 3)
        .reshape(NV // 2, 128, 2 * DIM)
    )
    wpj_t = np.ascontiguousarray(wpj_t)

    return {"xt": xt, "g": gb, "wup": wup_t, "wdn": wdn_t, "wpj": wpj_t}


_last_results = None  # for test harness inspection (exec_time_ns etc.)


def kernel(x, w_router, w_up, w_down, w_proj):
    global _last_results
    x = np.asarray(x, np.float32)

    ids, wsm = _route(x, w_router)
    ids_flat = ids.reshape(-1).astype(np.int64)
    w_flat = wsm.reshape(-1).astype(np.float32)
    order = np.argsort(ids_flat, kind="stable")
    counts = np.bincount(ids_flat, minlength=N_EXPERTS)
    offs = np.concatenate([[0], np.cumsum(counts)])

    C = int(counts.max())
    n_ch = max(1, -(-C // 512))
    base, rem = divmod(C, n_ch)
    sizes = [base + (1 if i < rem else 0) for i in range(n_ch)]
    chunks = []
    o = 0
    for s_ in sizes:
        chunks.append((o, o + s_))
        o += s_

    x_flat = x.reshape(B * S, DIM)
    in_maps = []
    tok_lists = []
    for e in range(N_EXPERTS):
        rows = order[offs[e] : offs[e + 1]]
        tok = rows // K
        tok_lists.append(tok)
        in_maps.append(
            _prep_core_inputs(e, x_flat, w_up, w_down, w_proj, tok, w_flat[rows], C)
        )

    nc = _build(C, chunks)

    from concourse.bass_utils import run_bass_kernel_spmd

    trace = bool(int(os.environ.get("MOE_KERNEL_TRACE", "0")))
    kw = {}
    if trace:
        kw["trace"] = True
        kw["trace_cores"] = list(range(N_CORES))
    res = run_bass_kernel_spmd(nc, in_maps, list(range(N_CORES)), **kw)
    _last_results = res

    out_flat = np.zeros((B * S, VOCAB), np.float32)
    for e in range(N_EXPERTS):
        tok = tok_lists[e]
        cnt = len(tok)
        if cnt:
            out_flat[tok] += res.results[e]["L"][:, :cnt].T.astype(np.float32)
    return out_flat.reshape(B, S, VOCAB)
